# revision 40
# baseline (speedup 1.0000x reference)
"""Trainium2 Bass kernel for nn_DynamicEmbedder (routed embedding + projection).

Reference computation (fp32):
    is_high = node_ids < 100_000
    out[b]  = is_high ? emb_high_w[id] @ W_high.T + b_high
                      : emb_low_w[id - 100_000] @ W_low.T + b_low

v4 strategy (8 NeuronCores): host routing/dedup/gather (as v2) + a
weights-stationary streaming GEMM in fp8-e3m4 end to end.

  * Host dedups ids (~79% distinct), gathers distinct rows, pre-scales
    them by 2^SCALE (exact), casts to fp8 e3m4 (1-3-4: rel err 2^-5,
    normals [2^-2, 15.5] -- ideal for the xavier-uniform data), and lays
    them out feature-major so the device streams them as matmul rhs.
  * Device keeps W resident in SBUF as the stationary lhsT:
      lo: two zero-padded K=128 lhsT matrices (W_low.T on complementary
          partition halves) project the two stacked row streams; full-
          partition rhs A/B'd 1.5 us faster than base-64 row-group rhs.
      hi: W_high.T split in two K=128 chunks accumulated in PSUM.
    PSUM holds out*2^SCALE; max |psum| ~ 13.7 (lo) / 28.7 (hi).
  * PSUM -> SBUF copies cast straight to e3m4 (lo) / bf16 (hi); the
    2^-SCALE correction folds into the host decode.  These copies ARE
    the bottleneck: f32-source copies run 1 elem/cycle/partition on
    both DVE (0.96 GHz) and ACT (1.2 GHz) -- every output element must
    cross one of them, ~27 us/core floor.  FD-1024 copy blocks from a
    shared 4-deep ring of 2-bank PSUM tiles keep both engines ~80%
    busy; DMA issuance lives on SP (loads) and Pool/SWDGE (stores).
  * Numerically validated offline AND bit-identical on device:
    rel err 1.2065e-2 vs the 2e-2 gate.
  * HBM traffic per core ~8.8 MB (was 18.6 MB in v2): in 2.9 (lo e3m4)
    + 1.3 (hi e3m4) + 0.13 (w) ; out 2.9 (lo e3m4) + 1.3 (hi bf16).
  * Chunk sizes ramp 512->4096->512 so the first copy starts ~3 us in
    and the final store is small (short drain).
  * Measured: ~33 us R-loop slope (v2 baseline: ~57 us in the same
    environment); CoreSim single-shot 34.0 us.  Machine-level drift of
    +-10% between processes -- compare variants via ab_test.py
    (interleaved in one process), not across runs.
"""

import sys

import numpy as np

for _p in ("/opt/trn_rl_repo", "/opt/pypackages"):
    if _p not in sys.path:
        sys.path.append(_p)

import ml_dtypes

import concourse.bass as bass  # noqa: F401
import concourse.mybir as mybir
import concourse.tile as tile
from concourse import bacc
from concourse.bass_utils import run_bass_kernel_spmd

# Problem constants (hardcoded per the harness contract).
NUM_NODES = 1_000_000
NUM_HIGH = 100_000
NUM_LOW = NUM_NODES - NUM_HIGH
D_HIGH, D_LOW, D_OUT = 256, 64, 128
BATCH = 500_000
N_CORES = 8

P = 128
NBLK = 512        # rows per matmul / PSUM bank
CPBLK = 2 * NBLK  # rows per PSUM->SBUF copy (2-bank PSUM tiles)
PAD_LO = 1024     # nl2 padding granule (nl_c multiple of 2*PAD_LO)
PAD_HI = 1024     # nh_c padding granule
CH_LO = 4096      # max lo rows per half-stream per chunk
CH_HI = 2048      # max hi rows per chunk
WARMUP_MMS = 3    # PE warm-up burst sized to hide inside the first
                  # input-load latency window (~1.7 us at cold rate)
REP_WARM_MMS = 0  # dead matmuls at each For_i rep start (A/B: no gain)
OUT_LO_E3M4 = 1   # 0: bf16 lo-output (2x store bytes, probes whether HW
                  # charges 8-bit-dst copies more than the model's 1x)
LO_K128 = 1       # zero-padded K=128 full-partition lo matmuls: A/B'd
                  # 1.5 us faster than base-partition-64 rhs streams
MM_GROUPED = 1    # g-major lo matmul order (same lhsT for consecutive MMs)
BUFS_IN = 3       # input-tile buffering depth
BUFS_OUT = 3      # output-staging depth
PS_BUFS = 4       # PSUM ring: 4 * 2 banks = 8 banks (shared lo/hi)
COPY_MOD = 2      # copy goes to ACT when ctr % COPY_MOD == COPY_MOD - 1
SCALE_LO = 11     # emb_low * 2^11: max ~5.3; psum max ~13.7 (e3m4 top 15.5)
SCALE_HI = 10     # emb_high * 2^10: max ~7.9; psum max ~28.7 (bf16 out)

F32 = mybir.dt.float32
BF16 = mybir.dt.bfloat16
E3M4 = mybir.dt.float8e3
NP_BF16 = np.dtype(ml_dtypes.bfloat16)
NP_E3M4 = np.dtype(ml_dtypes.float8_e3m4)


def _round_up(x, m):
    return ((x + m - 1) // m) * m


def _build_program(nh_cols, nl2_cols, n_reps=1):
    """Single-core SPMD-replicated streaming-GEMM program.

    nh_cols: padded high rows per core (multiple of CH_HI).
    nl2_cols: padded low rows per half-stream (multiple of CPBLK);
              total low rows per core = 2 * nl2_cols.
    """
    assert nh_cols % CPBLK == 0 and nl2_cols % CPBLK == 0

    nc = bacc.Bacc(
        "TRN2",
        target_bir_lowering=False,
        debug=False,
        enable_asserts=False,
        num_devices=N_CORES,
    )

    lo_out_dt = E3M4 if OUT_LO_E3M4 else BF16
    xt_lo = nc.dram_tensor("xt_lo", [P, nl2_cols], E3M4, kind="ExternalInput")
    xt_hi = nc.dram_tensor("xt_hi", [P, 2, nh_cols], E3M4,
                           kind="ExternalInput")
    w_lo = nc.dram_tensor("w_lo", [P, 2, D_OUT] if LO_K128 else [P, D_OUT],
                          BF16, kind="ExternalInput")
    w_hi = nc.dram_tensor("w_hi", [P, 2, D_OUT], BF16, kind="ExternalInput")
    out_lo = nc.dram_tensor("out_lo", [P, 2, nl2_cols], lo_out_dt,
                            kind="ExternalOutput")
    out_hi = nc.dram_tensor("out_hi", [P, nh_cols], BF16, kind="ExternalOutput")
    warm_out = nc.dram_tensor("warm_out", [P, 64], BF16, kind="ExternalOutput")

    from contextlib import ExitStack

    with tile.TileContext(nc) as tc, ExitStack() as ctx:
        const_pool = ctx.enter_context(tc.tile_pool(name="const", bufs=1))
        in_lo_pool = ctx.enter_context(tc.tile_pool(name="inlo", bufs=BUFS_IN))
        in_hi_pool = ctx.enter_context(tc.tile_pool(name="inhi", bufs=BUFS_IN))
        out_lo_pool = ctx.enter_context(tc.tile_pool(name="outlo", bufs=BUFS_OUT))
        out_hi_pool = ctx.enter_context(tc.tile_pool(name="outhi", bufs=BUFS_OUT))
        ps_pool = ctx.enter_context(tc.tile_pool(name="ps", bufs=PS_BUFS,
                                                 space="PSUM"))

        # Stationary weights: lo = W_low.T duplicated on both partition
        # halves ([128, 128]); hi = W_high.T as two K=128 chunks.
        w_lo_sb = const_pool.tile([P, 2, D_OUT] if LO_K128 else [P, D_OUT],
                                  BF16, tag="w_lo")
        w_hi_sb = const_pool.tile([P, 2, D_OUT], BF16, tag="w_hi")

        def load_weights():
            nc.sync.dma_start(w_lo_sb[:], w_lo.ap())
            nc.sync.dma_start(w_hi_sb[:], w_hi.ap())

        warm_rhs = const_pool.tile([P, NBLK], BF16, tag="warm_rhs")

        def warm_mms(n_mms):
            # Dead matmuls on the zeroed tile: no data dependency, so the
            # PE can run them while input DMAs are still in flight.  Used
            # in the prologue and at the top of each For_i rep (the
            # iteration barrier idles the PE > the 3.4us HAM MID window,
            # re-throttling it to 1.2 GHz).
            warm_tile = ps_pool.tile([P, 2, NBLK], F32, tag="ps")
            warm_ps = warm_tile[:, 0, :]
            for _ in range(n_mms):
                nc.tensor.matmul(warm_ps[:], lhsT=warm_rhs[:, 0:P],
                                 rhs=warm_rhs[:], start=True, stop=True,
                                 skip_group_check=True)
            return warm_ps

        def warmup(n_mms):
            nc.vector.memset(warm_rhs[:], 0.0)
            warm_ps = warm_mms(n_mms)
            warm_sb = const_pool.tile([P, 64], BF16, tag="warm_sb")
            nc.scalar.copy(warm_sb[:], warm_ps[:, 0:64])
            nc.sync.dma_start(warm_out.ap(), warm_sb[:])

        # DVE copy = 1192 ns, ACT = 1022 ns per FD-1024 block (both 1x:
        # f32 PSUM source reads 1 elem/cycle); balance ~23/26.
        cctr = [0]

        def do_copy(dst, src):
            cctr[0] += 1
            if cctr[0] % 15 % 2 == 1:
                nc.vector.tensor_copy(dst, src)
            else:
                nc.scalar.copy(dst, src)

        def do_lo_chunk(c0, clen, in_pre=None):
            if in_pre is None:
                in_sb = in_lo_pool.tile([P, clen], E3M4, tag="in")
                nc.sync.dma_start(in_sb[:], xt_lo.ap()[:, c0:c0 + clen])
            else:
                in_sb = in_pre
            out_sb = out_lo_pool.tile([P, 2, clen], lo_out_dt, tag="out")
            blocks = clen // NBLK
            # g-major: consecutive matmuls share the same stationary lhsT
            # (fewer LDWEIGHTS switches) when MM_GROUPED.
            hg = ([(h, g) for g in range(2) for h in range(0, blocks, 2)]
                  if MM_GROUPED else
                  [(h, g) for h in range(0, blocks, 2) for g in range(2)])
            for h, g in hg:
                k = min(2, blocks - h)
                sl = slice(h * NBLK, (h + k) * NBLK)
                gsl = slice(g * 64, (g + 1) * 64)
                ps = ps_pool.tile([P, 2, NBLK], F32, tag="ps")
                for q in range(k):
                    qsl = slice((h + q) * NBLK, (h + q + 1) * NBLK)
                    if LO_K128:
                        nc.tensor.matmul(ps[:, q, :],
                                         lhsT=w_lo_sb[:, g, :],
                                         rhs=in_sb[:, qsl],
                                         start=True, stop=True,
                                         skip_group_check=True)
                    else:
                        nc.tensor.matmul(ps[:, q, :],
                                         lhsT=w_lo_sb[gsl, :],
                                         rhs=in_sb[gsl, qsl],
                                         start=True, stop=True,
                                         skip_group_check=True)
                do_copy(out_sb[:, g, sl], ps[:, 0:k, :])
            nc.gpsimd.dma_start(out_lo.ap()[:, :, c0:c0 + clen], out_sb[:])

        def do_hi_chunk(c0, clen):
            in_sb = in_hi_pool.tile([P, 2, clen], E3M4, tag="in")
            nc.sync.dma_start(in_sb[:], xt_hi.ap()[:, :, c0:c0 + clen])
            out_sb = out_hi_pool.tile([P, clen], BF16, tag="out")
            blocks = clen // NBLK
            for h in range(0, blocks, 2):
                k = min(2, blocks - h)
                sl = slice(h * NBLK, (h + k) * NBLK)
                ps = ps_pool.tile([P, 2, NBLK], F32, tag="ps")
                for q in range(k):
                    qsl = slice((h + q) * NBLK, (h + q + 1) * NBLK)
                    for cchunk in range(2):
                        nc.tensor.matmul(ps[:, q, :],
                                         lhsT=w_hi_sb[:, cchunk, :],
                                         rhs=in_sb[:, cchunk, qsl],
                                         start=(cchunk == 0),
                                         stop=(cchunk == 1),
                                         skip_group_check=True)
                do_copy(out_sb[:, sl], ps[:, 0:k, :])
            nc.gpsimd.dma_start(out_hi.ap()[:, c0:c0 + clen], out_sb[:])

        def ramp_sizes(total, ch):
            """Chunk sizes: small at both ends (short ramp/drain), ch mid."""
            sizes, tail = [], []
            rem = total
            for s in (512, 1024, 2048):
                if s < ch and rem >= s + 3584:
                    sizes.append(s)
                    rem -= s
            for s in (512, 1024, 2048):
                if s < ch and rem >= s:
                    tail.append(s)
                    rem -= s
            while rem > 0:
                c = min(ch, rem)
                sizes.append(c)
                rem -= c
            return sizes + tail[::-1]

        lo_sizes = ramp_sizes(nl2_cols, CH_LO)
        hi_sizes = ramp_sizes(nh_cols, CH_HI)

        def body(first_in=None):
            jobs = []
            pos = 0
            for i, sz in enumerate(lo_sizes):
                jobs.append(((pos + sz / 2) / nl2_cols, 0, do_lo_chunk, pos, sz,
                             first_in if i == 0 else None))
                pos += sz
            pos = 0
            for i, sz in enumerate(hi_sizes):
                jobs.append(((pos + sz / 2) / nh_cols, 1, do_hi_chunk, pos, sz,
                             None))
                pos += sz
            jobs.sort(key=lambda t: (t[0], t[1]))
            for _, _, fn, c0, sz, pre in jobs:
                if pre is not None:
                    fn(c0, sz, pre)
                else:
                    fn(c0, sz)

        if n_reps == 1:
            # Hoist chunk 0's input load ahead of the weight loads so the
            # first matmuls start ~1.5 us earlier (prologue only).
            first_in = in_lo_pool.tile([P, lo_sizes[0]], E3M4, tag="in")
            nc.sync.dma_start(first_in[:], xt_lo.ap()[:, 0:lo_sizes[0]])
            load_weights()
            if WARMUP_MMS:
                warmup(WARMUP_MMS)
            body(first_in)
        else:
            load_weights()
            if WARMUP_MMS:
                warmup(WARMUP_MMS)
            with tc.For_i(0, n_reps, 1):
                if REP_WARM_MMS:
                    warm_mms(REP_WARM_MMS)
                body()

    nc.compile()
    return nc


_PROGRAM_CACHE = {}


def _get_program(nh_cols, nl2_cols, n_reps=1):
    key = (nh_cols, nl2_cols, n_reps)
    if key not in _PROGRAM_CACHE:
        _PROGRAM_CACHE[key] = _build_program(nh_cols, nl2_cols, n_reps=n_reps)
    return _PROGRAM_CACHE[key]


def _route(node_ids):
    """Dedup ids and compute per-core padded section sizes."""
    ids = np.asarray(node_ids).astype(np.int64)
    uniq, inv = np.unique(ids, return_inverse=True)
    nh_tot = int(np.searchsorted(uniq, NUM_HIGH))
    uniq_hi = uniq[:nh_tot]
    uniq_lo = uniq[nh_tot:] - NUM_HIGH
    nl_tot = uniq_lo.size
    nh_c = _round_up(max(1, -(-nh_tot // N_CORES)), PAD_HI)
    nl_c = _round_up(max(2, -(-nl_tot // N_CORES)), 2 * PAD_LO)
    return uniq_hi, uniq_lo, inv, nh_tot, nl_tot, nh_c, nl_c


def _make_in_maps(route, emb_high_w, emb_low_w, W_high, W_low):
    uniq_hi, uniq_lo, inv, nh_tot, nl_tot, nh_c, nl_c = route
    emb_high_w = np.asarray(emb_high_w, np.float32)
    emb_low_w = np.asarray(emb_low_w, np.float32)

    wloT = np.asarray(W_low, np.float32).T            # [64, 128]
    if LO_K128:
        wlo = np.zeros((P, 2, D_OUT), np.float32)
        wlo[0:64, 0, :] = wloT
        wlo[64:128, 1, :] = wloT
        wlo = wlo.astype(NP_BF16)
    else:
        wlo = np.concatenate([wloT, wloT], axis=0).astype(NP_BF16)
    whiT = np.ascontiguousarray(
        np.asarray(W_high, np.float32).T.reshape(2, P, D_OUT)
        .transpose(1, 0, 2)).astype(NP_BF16)          # [128, 2, 128]

    uh_pad = np.zeros(N_CORES * nh_c, np.int64)
    uh_pad[:nh_tot] = uniq_hi
    uh_pad = uh_pad.reshape(N_CORES, nh_c)
    ul_pad = np.zeros(N_CORES * nl_c, np.int64)
    ul_pad[:nl_tot] = uniq_lo
    ul_pad = ul_pad.reshape(N_CORES, nl_c)

    nl2 = nl_c // 2
    in_maps = []
    for c in range(N_CORES):
        Xh = emb_high_w[uh_pad[c]] * np.float32(2.0 ** SCALE_HI)
        xt_hi = np.ascontiguousarray(
            Xh.reshape(nh_c, 2, P).transpose(2, 1, 0)).astype(NP_E3M4)
        Xl = (emb_low_w[ul_pad[c]] * np.float32(2.0 ** SCALE_LO)) \
            .astype(NP_E3M4)                          # [nl_c, 64]
        xt_lo = np.concatenate([Xl[:nl2].T, Xl[nl2:].T], axis=0)  # [128, nl2]
        in_maps.append({
            "xt_lo": np.ascontiguousarray(xt_lo),
            "xt_hi": xt_hi,
            "w_lo": wlo,
            "w_hi": whiT,
        })
    return in_maps


def _decode(results, route, b_high, b_low):
    uniq_hi, uniq_lo, inv, nh_tot, nl_tot, nh_c, nl_c = route
    nl2 = nl_c // 2
    rowout = np.empty((nh_tot + nl_tot, D_OUT), np.float32)
    for c in range(N_CORES):
        cnt = min(max(nh_tot - c * nh_c, 0), nh_c)
        if cnt:
            r = np.asarray(results[c]["out_hi"])      # [128, nh_c] bf16
            dec = r.T.astype(np.float32) * np.float32(2.0 ** -SCALE_HI)
            rowout[c * nh_c:c * nh_c + cnt] = dec[:cnt]
        cnt = min(max(nl_tot - c * nl_c, 0), nl_c)
        if cnt:
            r = np.asarray(results[c]["out_lo"])      # [128, 2, nl2] e3m4
            dec = r.reshape(P, nl_c).T.astype(np.float32) \
                * np.float32(2.0 ** -SCALE_LO)
            rowout[nh_tot + c * nl_c:nh_tot + c * nl_c + cnt] = dec[:cnt]
    rowout[:nh_tot] += np.asarray(b_high, np.float32)
    rowout[nh_tot:] += np.asarray(b_low, np.float32)
    return rowout[inv]


def _prepare(inputs):
    """(nc, in_maps) for external profiling harnesses."""
    route = _route(inputs["node_ids"])
    nc = _get_program(route[5], route[6] // 2)
    in_maps = _make_in_maps(route, inputs["emb_high_w"], inputs["emb_low_w"],
                            inputs["W_high"], inputs["W_low"])
    return nc, in_maps


def kernel(node_ids, emb_high_w, emb_low_w, W_high, b_high, W_low, b_low):
    route = _route(node_ids)
    nh_c, nl_c = route[5], route[6]
    nc = _get_program(nh_c, nl_c // 2)
    in_maps = _make_in_maps(route, emb_high_w, emb_low_w, W_high, W_low)
    res = run_bass_kernel_spmd(nc, in_maps, core_ids=list(range(N_CORES)))
    return _decode(res.results, route, b_high, b_low)


# revision 43
# speedup vs baseline: 1.0367x; 1.0367x over previous
"""Trainium2 Bass kernel for nn_DynamicEmbedder (routed embedding + projection).

Reference computation (fp32):
    is_high = node_ids < 100_000
    out[b]  = is_high ? emb_high_w[id] @ W_high.T + b_high
                      : emb_low_w[id - 100_000] @ W_low.T + b_low

v4 strategy (8 NeuronCores): host routing/dedup/gather (as v2) + a
weights-stationary streaming GEMM in fp8-e3m4 end to end.

  * Host dedups ids (~79% distinct), gathers distinct rows, pre-scales
    them by 2^SCALE (exact), casts to fp8 e3m4 (1-3-4: rel err 2^-5,
    normals [2^-2, 15.5] -- ideal for the xavier-uniform data), and lays
    them out feature-major so the device streams them as matmul rhs.
  * Device keeps W resident in SBUF as the stationary lhsT:
      lo: two zero-padded K=128 lhsT matrices (W_low.T on complementary
          partition halves) project the two stacked row streams; full-
          partition rhs A/B'd 1.5 us faster than base-64 row-group rhs.
      hi: W_high.T split in two K=128 chunks accumulated in PSUM.
    PSUM holds out*2^SCALE; max |psum| ~ 13.7 (lo) / 28.7 (hi).
  * PSUM -> SBUF copies cast straight to e3m4 (lo) / bf16 (hi); the
    2^-SCALE correction folds into the host decode.  These copies ARE
    the bottleneck: f32-source copies run 1 elem/cycle/partition on
    both DVE (0.96 GHz) and ACT (1.2 GHz) -- every output element must
    cross one of them, ~27 us/core floor.  FD-1024 copy blocks from a
    shared 4-deep ring of 2-bank PSUM tiles keep both engines ~80%
    busy; DMA issuance lives on SP (loads) and Pool/SWDGE (stores).
  * Numerically validated offline AND bit-identical on device:
    rel err 1.2065e-2 vs the 2e-2 gate.
  * HBM traffic per core ~8.8 MB (was 18.6 MB in v2): in 2.9 (lo e3m4)
    + 1.3 (hi e3m4) + 0.13 (w) ; out 2.9 (lo e3m4) + 1.3 (hi bf16).
  * Chunk sizes ramp 512->4096->512 so the first copy starts ~3 us in
    and the final store is small (short drain).
  * Measured: ~33 us R-loop slope (v2 baseline: ~57 us in the same
    environment); CoreSim single-shot 34.0 us.  Machine-level drift of
    +-10% between processes -- compare variants via ab_test.py
    (interleaved in one process), not across runs.
"""

import sys

import numpy as np

for _p in ("/opt/trn_rl_repo", "/opt/pypackages"):
    if _p not in sys.path:
        sys.path.append(_p)

import ml_dtypes

import concourse.bass as bass  # noqa: F401
import concourse.mybir as mybir
import concourse.tile as tile
from concourse import bacc
from concourse.bass_utils import run_bass_kernel_spmd

# Problem constants (hardcoded per the harness contract).
NUM_NODES = 1_000_000
NUM_HIGH = 100_000
NUM_LOW = NUM_NODES - NUM_HIGH
D_HIGH, D_LOW, D_OUT = 256, 64, 128
BATCH = 500_000
N_CORES = 8

P = 128
NBLK = 512        # rows per matmul / PSUM bank
CPBLK = 2 * NBLK  # rows per PSUM->SBUF copy (2-bank PSUM tiles)
PAD_LO = 1024     # nl2 padding granule (nl_c multiple of 2*PAD_LO)
PAD_HI = 1024     # nh_c padding granule
CH_LO = 4096      # max lo rows per half-stream per chunk
CH_HI = 2048      # max hi rows per chunk
WARMUP_MMS = 3    # PE warm-up burst sized to hide inside the first
                  # input-load latency window (~1.7 us at cold rate)
REP_WARM_MMS = 0  # dead matmuls at each For_i rep start (A/B: no gain)
OUT_LO_E3M4 = 1   # 0: bf16 lo-output (2x store bytes, probes whether HW
                  # charges 8-bit-dst copies more than the model's 1x)
LO_K128 = 1       # zero-padded K=128 full-partition lo matmuls: A/B'd
                  # 1.5 us faster than base-partition-64 rhs streams
MM_GROUPED = 1    # g-major lo matmul order (same lhsT for consecutive MMs)
COPY_PAT = 0      # DVE share of copies: 0 -> 7/15, 1 -> 1/2, 2 -> 2/5
PRELOOP = 0       # loop-carried chunk-0 preload buffer (R-loop only):
                  # body ends by reloading it for the next rep, so post-
                  # barrier matmuls start without waiting for a load
BUFS_IN = 3       # input-tile buffering depth
BUFS_OUT = 3      # output-staging depth
PS_BUFS = 4       # PSUM ring: 4 * 2 banks = 8 banks (shared lo/hi)
COPY_MOD = 2      # copy goes to ACT when ctr % COPY_MOD == COPY_MOD - 1
SCALE_LO = 11     # emb_low * 2^11: max ~5.3; psum max ~13.7 (e3m4 top 15.5)
SCALE_HI = 10     # emb_high * 2^10: max ~7.9; psum max ~28.7 (bf16 out)

F32 = mybir.dt.float32
BF16 = mybir.dt.bfloat16
E3M4 = mybir.dt.float8e3
NP_BF16 = np.dtype(ml_dtypes.bfloat16)
NP_E3M4 = np.dtype(ml_dtypes.float8_e3m4)


def _round_up(x, m):
    return ((x + m - 1) // m) * m


def _build_program(nh_cols, nl2_cols, n_reps=1):
    """Single-core SPMD-replicated streaming-GEMM program.

    nh_cols: padded high rows per core (multiple of CH_HI).
    nl2_cols: padded low rows per half-stream (multiple of CPBLK);
              total low rows per core = 2 * nl2_cols.
    """
    assert nh_cols % CPBLK == 0 and nl2_cols % CPBLK == 0

    nc = bacc.Bacc(
        "TRN2",
        target_bir_lowering=False,
        debug=False,
        enable_asserts=False,
        num_devices=N_CORES,
    )

    lo_out_dt = E3M4 if OUT_LO_E3M4 else BF16
    xt_lo = nc.dram_tensor("xt_lo", [P, nl2_cols], E3M4, kind="ExternalInput")
    xt_hi = nc.dram_tensor("xt_hi", [P, 2, nh_cols], E3M4,
                           kind="ExternalInput")
    w_lo = nc.dram_tensor("w_lo", [P, 2, D_OUT] if LO_K128 else [P, D_OUT],
                          BF16, kind="ExternalInput")
    w_hi = nc.dram_tensor("w_hi", [P, 2, D_OUT], BF16, kind="ExternalInput")
    out_lo = nc.dram_tensor("out_lo", [P, 2, nl2_cols], lo_out_dt,
                            kind="ExternalOutput")
    out_hi = nc.dram_tensor("out_hi", [P, nh_cols], BF16, kind="ExternalOutput")
    warm_out = nc.dram_tensor("warm_out", [P, 64], BF16, kind="ExternalOutput")

    from contextlib import ExitStack

    with tile.TileContext(nc) as tc, ExitStack() as ctx:
        const_pool = ctx.enter_context(tc.tile_pool(name="const", bufs=1))
        in_lo_pool = ctx.enter_context(tc.tile_pool(name="inlo", bufs=BUFS_IN))
        in_hi_pool = ctx.enter_context(tc.tile_pool(name="inhi", bufs=BUFS_IN))
        out_lo_pool = ctx.enter_context(tc.tile_pool(name="outlo", bufs=BUFS_OUT))
        out_hi_pool = ctx.enter_context(tc.tile_pool(name="outhi", bufs=BUFS_OUT))
        ps_pool = ctx.enter_context(tc.tile_pool(name="ps", bufs=PS_BUFS,
                                                 space="PSUM"))

        # Stationary weights: lo = W_low.T duplicated on both partition
        # halves ([128, 128]); hi = W_high.T as two K=128 chunks.
        w_lo_sb = const_pool.tile([P, 2, D_OUT] if LO_K128 else [P, D_OUT],
                                  BF16, tag="w_lo")
        w_hi_sb = const_pool.tile([P, 2, D_OUT], BF16, tag="w_hi")

        def load_weights():
            nc.sync.dma_start(w_lo_sb[:], w_lo.ap())
            nc.sync.dma_start(w_hi_sb[:], w_hi.ap())

        warm_rhs = const_pool.tile([P, NBLK], BF16, tag="warm_rhs")

        def warm_mms(n_mms):
            # Dead matmuls on the zeroed tile: no data dependency, so the
            # PE can run them while input DMAs are still in flight.  Used
            # in the prologue and at the top of each For_i rep (the
            # iteration barrier idles the PE > the 3.4us HAM MID window,
            # re-throttling it to 1.2 GHz).
            warm_tile = ps_pool.tile([P, 2, NBLK], F32, tag="ps")
            warm_ps = warm_tile[:, 0, :]
            for _ in range(n_mms):
                nc.tensor.matmul(warm_ps[:], lhsT=warm_rhs[:, 0:P],
                                 rhs=warm_rhs[:], start=True, stop=True,
                                 skip_group_check=True)
            return warm_ps

        def warmup(n_mms):
            nc.vector.memset(warm_rhs[:], 0.0)
            warm_ps = warm_mms(n_mms)
            warm_sb = const_pool.tile([P, 64], BF16, tag="warm_sb")
            nc.scalar.copy(warm_sb[:], warm_ps[:, 0:64])
            nc.sync.dma_start(warm_out.ap(), warm_sb[:])

        # DVE copy = 1192 ns, ACT = 1022 ns per FD-1024 block (both 1x:
        # f32 PSUM source reads 1 elem/cycle); balance ~23/26.
        cctr = [0]

        def do_copy(dst, src):
            cctr[0] += 1
            if COPY_PAT == 0:
                dve = cctr[0] % 15 % 2 == 1
            elif COPY_PAT == 1:
                dve = cctr[0] % 2 == 1
            else:
                dve = cctr[0] % 5 in (1, 3)
            if dve:
                nc.vector.tensor_copy(dst, src)
            else:
                nc.scalar.copy(dst, src)

        def do_lo_chunk(c0, clen, in_pre=None, store=None):
            if in_pre is None:
                in_sb = in_lo_pool.tile([P, clen], E3M4, tag="in")
                nc.sync.dma_start(in_sb[:], xt_lo.ap()[:, c0:c0 + clen])
            else:
                in_sb = in_pre
            out_sb = out_lo_pool.tile([P, 2, clen], lo_out_dt, tag="out")
            blocks = clen // NBLK
            # g-major: consecutive matmuls share the same stationary lhsT
            # (fewer LDWEIGHTS switches) when MM_GROUPED.
            hg = ([(h, g) for g in range(2) for h in range(0, blocks, 2)]
                  if MM_GROUPED else
                  [(h, g) for h in range(0, blocks, 2) for g in range(2)])
            for h, g in hg:
                k = min(2, blocks - h)
                sl = slice(h * NBLK, (h + k) * NBLK)
                gsl = slice(g * 64, (g + 1) * 64)
                ps = ps_pool.tile([P, 2, NBLK], F32, tag="ps")
                for q in range(k):
                    qsl = slice((h + q) * NBLK, (h + q + 1) * NBLK)
                    if LO_K128:
                        nc.tensor.matmul(ps[:, q, :],
                                         lhsT=w_lo_sb[:, g, :],
                                         rhs=in_sb[:, qsl],
                                         start=True, stop=True,
                                         skip_group_check=True)
                    else:
                        nc.tensor.matmul(ps[:, q, :],
                                         lhsT=w_lo_sb[gsl, :],
                                         rhs=in_sb[gsl, qsl],
                                         start=True, stop=True,
                                         skip_group_check=True)
                do_copy(out_sb[:, g, sl], ps[:, 0:k, :])
            (store or nc.gpsimd.dma_start)(
                out_lo.ap()[:, :, c0:c0 + clen], out_sb[:])

        def do_hi_chunk(c0, clen, store=None):
            in_sb = in_hi_pool.tile([P, 2, clen], E3M4, tag="in")
            nc.sync.dma_start(in_sb[:], xt_hi.ap()[:, :, c0:c0 + clen])
            out_sb = out_hi_pool.tile([P, clen], BF16, tag="out")
            blocks = clen // NBLK
            for h in range(0, blocks, 2):
                k = min(2, blocks - h)
                sl = slice(h * NBLK, (h + k) * NBLK)
                ps = ps_pool.tile([P, 2, NBLK], F32, tag="ps")
                for q in range(k):
                    qsl = slice((h + q) * NBLK, (h + q + 1) * NBLK)
                    for cchunk in range(2):
                        nc.tensor.matmul(ps[:, q, :],
                                         lhsT=w_hi_sb[:, cchunk, :],
                                         rhs=in_sb[:, cchunk, qsl],
                                         start=(cchunk == 0),
                                         stop=(cchunk == 1),
                                         skip_group_check=True)
                do_copy(out_sb[:, sl], ps[:, 0:k, :])
            (store or nc.gpsimd.dma_start)(
                out_hi.ap()[:, c0:c0 + clen], out_sb[:])

        def ramp_sizes(total, ch):
            """Chunk sizes: small at both ends (short ramp/drain), ch mid."""
            sizes, tail = [], []
            rem = total
            for s in (512, 1024, 2048):
                if s < ch and rem >= s + 3584:
                    sizes.append(s)
                    rem -= s
            for s in (512, 1024, 2048):
                if s < ch and rem >= s:
                    tail.append(s)
                    rem -= s
            while rem > 0:
                c = min(ch, rem)
                sizes.append(c)
                rem -= c
            return sizes + tail[::-1]

        lo_sizes = ramp_sizes(nl2_cols, CH_LO)
        hi_sizes = ramp_sizes(nh_cols, CH_HI)

        def body(first_in=None):
            jobs = []
            pos = 0
            for i, sz in enumerate(lo_sizes):
                jobs.append(((pos + sz / 2) / nl2_cols, 0, do_lo_chunk, pos, sz,
                             first_in if i == 0 else None))
                pos += sz
            pos = 0
            for i, sz in enumerate(hi_sizes):
                jobs.append(((pos + sz / 2) / nh_cols, 1, do_hi_chunk, pos, sz,
                             None))
                pos += sz
            jobs.sort(key=lambda t: (t[0], t[1]))
            for p, _, fn, c0, sz, pre in jobs:
                # Tail stores go via SP/HWDGE (~0.4us lower first-byte
                # latency than SWDGE, and SP is idle once loads finish)
                # to shorten the end-of-body drain.
                store = nc.sync.dma_start if p > 0.9 else None
                if pre is not None:
                    fn(c0, sz, pre, store=store)
                else:
                    fn(c0, sz, store=store)

        if n_reps == 1:
            # Hoist chunk 0's input load ahead of the weight loads so the
            # first matmuls start ~1.5 us earlier (prologue only).
            first_in = in_lo_pool.tile([P, lo_sizes[0]], E3M4, tag="in")
            nc.sync.dma_start(first_in[:], xt_lo.ap()[:, 0:lo_sizes[0]])
            load_weights()
            if WARMUP_MMS:
                warmup(WARMUP_MMS)
            body(first_in)
        elif PRELOOP:
            pre_in = const_pool.tile([P, lo_sizes[0]], E3M4, tag="pre_in")
            nc.sync.dma_start(pre_in[:], xt_lo.ap()[:, 0:lo_sizes[0]])
            load_weights()
            if WARMUP_MMS:
                warmup(WARMUP_MMS)
            with tc.For_i(0, n_reps, 1):
                if REP_WARM_MMS:
                    warm_mms(REP_WARM_MMS)
                body(pre_in)
                # reload chunk 0 for the next rep (same data: timing loop);
                # WAR on pre_in orders this after this rep's chunk-0 MMs.
                nc.sync.dma_start(pre_in[:], xt_lo.ap()[:, 0:lo_sizes[0]])
        else:
            load_weights()
            if WARMUP_MMS:
                warmup(WARMUP_MMS)
            with tc.For_i(0, n_reps, 1):
                if REP_WARM_MMS:
                    warm_mms(REP_WARM_MMS)
                body()

    nc.compile()
    return nc


_PROGRAM_CACHE = {}


def _get_program(nh_cols, nl2_cols, n_reps=1):
    key = (nh_cols, nl2_cols, n_reps)
    if key not in _PROGRAM_CACHE:
        _PROGRAM_CACHE[key] = _build_program(nh_cols, nl2_cols, n_reps=n_reps)
    return _PROGRAM_CACHE[key]


def _route(node_ids):
    """Dedup ids and compute per-core padded section sizes."""
    ids = np.asarray(node_ids).astype(np.int64)
    uniq, inv = np.unique(ids, return_inverse=True)
    nh_tot = int(np.searchsorted(uniq, NUM_HIGH))
    uniq_hi = uniq[:nh_tot]
    uniq_lo = uniq[nh_tot:] - NUM_HIGH
    nl_tot = uniq_lo.size
    nh_c = _round_up(max(1, -(-nh_tot // N_CORES)), PAD_HI)
    nl_c = _round_up(max(2, -(-nl_tot // N_CORES)), 2 * PAD_LO)
    return uniq_hi, uniq_lo, inv, nh_tot, nl_tot, nh_c, nl_c


def _make_in_maps(route, emb_high_w, emb_low_w, W_high, W_low):
    uniq_hi, uniq_lo, inv, nh_tot, nl_tot, nh_c, nl_c = route
    emb_high_w = np.asarray(emb_high_w, np.float32)
    emb_low_w = np.asarray(emb_low_w, np.float32)

    wloT = np.asarray(W_low, np.float32).T            # [64, 128]
    if LO_K128:
        wlo = np.zeros((P, 2, D_OUT), np.float32)
        wlo[0:64, 0, :] = wloT
        wlo[64:128, 1, :] = wloT
        wlo = wlo.astype(NP_BF16)
    else:
        wlo = np.concatenate([wloT, wloT], axis=0).astype(NP_BF16)
    whiT = np.ascontiguousarray(
        np.asarray(W_high, np.float32).T.reshape(2, P, D_OUT)
        .transpose(1, 0, 2)).astype(NP_BF16)          # [128, 2, 128]

    uh_pad = np.zeros(N_CORES * nh_c, np.int64)
    uh_pad[:nh_tot] = uniq_hi
    uh_pad = uh_pad.reshape(N_CORES, nh_c)
    ul_pad = np.zeros(N_CORES * nl_c, np.int64)
    ul_pad[:nl_tot] = uniq_lo
    ul_pad = ul_pad.reshape(N_CORES, nl_c)

    nl2 = nl_c // 2
    in_maps = []
    for c in range(N_CORES):
        Xh = emb_high_w[uh_pad[c]] * np.float32(2.0 ** SCALE_HI)
        xt_hi = np.ascontiguousarray(
            Xh.reshape(nh_c, 2, P).transpose(2, 1, 0)).astype(NP_E3M4)
        Xl = (emb_low_w[ul_pad[c]] * np.float32(2.0 ** SCALE_LO)) \
            .astype(NP_E3M4)                          # [nl_c, 64]
        xt_lo = np.concatenate([Xl[:nl2].T, Xl[nl2:].T], axis=0)  # [128, nl2]
        in_maps.append({
            "xt_lo": np.ascontiguousarray(xt_lo),
            "xt_hi": xt_hi,
            "w_lo": wlo,
            "w_hi": whiT,
        })
    return in_maps


def _decode(results, route, b_high, b_low):
    uniq_hi, uniq_lo, inv, nh_tot, nl_tot, nh_c, nl_c = route
    nl2 = nl_c // 2
    rowout = np.empty((nh_tot + nl_tot, D_OUT), np.float32)
    for c in range(N_CORES):
        cnt = min(max(nh_tot - c * nh_c, 0), nh_c)
        if cnt:
            r = np.asarray(results[c]["out_hi"])      # [128, nh_c] bf16
            dec = r.T.astype(np.float32) * np.float32(2.0 ** -SCALE_HI)
            rowout[c * nh_c:c * nh_c + cnt] = dec[:cnt]
        cnt = min(max(nl_tot - c * nl_c, 0), nl_c)
        if cnt:
            r = np.asarray(results[c]["out_lo"])      # [128, 2, nl2] e3m4
            dec = r.reshape(P, nl_c).T.astype(np.float32) \
                * np.float32(2.0 ** -SCALE_LO)
            rowout[nh_tot + c * nl_c:nh_tot + c * nl_c + cnt] = dec[:cnt]
    rowout[:nh_tot] += np.asarray(b_high, np.float32)
    rowout[nh_tot:] += np.asarray(b_low, np.float32)
    return rowout[inv]


def _prepare(inputs):
    """(nc, in_maps) for external profiling harnesses."""
    route = _route(inputs["node_ids"])
    nc = _get_program(route[5], route[6] // 2)
    in_maps = _make_in_maps(route, inputs["emb_high_w"], inputs["emb_low_w"],
                            inputs["W_high"], inputs["W_low"])
    return nc, in_maps


def kernel(node_ids, emb_high_w, emb_low_w, W_high, b_high, W_low, b_low):
    route = _route(node_ids)
    nh_c, nl_c = route[5], route[6]
    nc = _get_program(nh_c, nl_c // 2)
    in_maps = _make_in_maps(route, emb_high_w, emb_low_w, W_high, W_low)
    res = run_bass_kernel_spmd(nc, in_maps, core_ids=list(range(N_CORES)))
    return _decode(res.results, route, b_high, b_low)
